# revision 18
# baseline (speedup 1.0000x reference)
"""Trainium2 Bass kernel for nn_Decoder2 (dense transformer decoder block).

Sharding (8 cores):
  - both attentions: head-sharded, 2 heads (=128 feature dims) per core
  - FFN: hidden dim column/row sharded, 512 hidden units per core; the 8
    partial outputs are summed on the host
  - wemb/pemb replicated; activations kept transposed [feat, seq]

v2 design (vs the 400us baseline):
  - everything bf16 (inputs, weights, activations); PSUM accumulates fp32.
  - wemb/pemb and ALL weights are SBUF-resident (loaded once at start) --
    no per-chunk input DMA, no per-chunk W2 reload.
  - V projections computed directly transposed ([keys, vdim] blocks) by
    using the x chunk as the stationary operand -- no PE transposes, no
    per-block ones-column copies (ones columns memset once).
  - per-chunk AllGathers (8 small ones instead of 4 big), FFN phase moved
    after the whole cross phase so every AG has >25us to complete before
    its consumer -- no PE stalls on collectives.
  - causal diagonal blocks trimmed: scores/exp only computed for live
    queries; AV runs per 128-query sub-block so diagonal j-steps skip
    dead sub-blocks (clean PSUM start/stop per sub-block).
  - causal masking via gpsimd.affine_select on the one partial sub-block
    (Pool engine, idle) instead of DVE mask multiplies.
  - FFN relu + output copies on the Scalar engine (idle during FFN).

Softmax without max-subtraction (scores are O(+-6)); denominator from a
ones-column folded into the AV lhsT (m=65). Scores for the two heads are
issued as adjacent K=64 row-tiles (tile_position) so they run concurrently
on the PE.
"""

import ml_dtypes
import numpy as np

import concourse.bass as bass
import concourse.bacc as bacc
import concourse.mybir as mybir
import concourse.tile as tile
from concourse.bass_utils import run_bass_kernel_spmd

F32 = mybir.dt.float32
BF16 = mybir.dt.bfloat16
AF = mybir.ActivationFunctionType

DEBUG_TAPS = False

N_CORES = 8
S_W, S_P = 2048, 1024
D_MODEL, NEW_DIM, H, D_FF = 1024, 1024, 16, 4096
HD = 128                  # head-feature dims per core (2 heads x 64)
FF_SH = D_FF // N_CORES   # 512 hidden units per core
NC = 512                  # free-dim chunk for matmuls
DCH = D_MODEL // 128      # 8 contraction chunks of 128
NSQ = S_W // NC           # 4 self (query) chunks
NSC = S_P // NC           # 2 pemb chunks
NSKB = S_W // 128         # 16 self key blocks
NSPB = S_P // 128         # 8 cross key blocks
NFB = FF_SH // 128        # 4 ffn hidden blocks per core


def decoder_kernel(tc):
    nc = tc.nc

    # all inputs host-prepacked to [128, ...] partition-major, bf16
    wembT = nc.dram_tensor("wembT", [128, NSQ * DCH * NC], BF16,
                           kind="ExternalInput").ap()
    pembT = nc.dram_tensor("pembT", [128, NSC * DCH * NC], BF16,
                           kind="ExternalInput").ap()
    wqmT = nc.dram_tensor("wqmT", [128, DCH * HD], BF16, kind="ExternalInput").ap()
    wkmT = nc.dram_tensor("wkmT", [128, DCH * HD], BF16, kind="ExternalInput").ap()
    wvmT = nc.dram_tensor("wvmT", [128, DCH * HD], BF16, kind="ExternalInput").ap()
    wqcT = nc.dram_tensor("wqcT", [128, DCH * HD], BF16, kind="ExternalInput").ap()
    wkcT = nc.dram_tensor("wkcT", [128, DCH * HD], BF16, kind="ExternalInput").ap()
    wvcT = nc.dram_tensor("wvcT", [128, DCH * HD], BF16, kind="ExternalInput").ap()
    w1T = nc.dram_tensor("w1T", [128, DCH * FF_SH], BF16, kind="ExternalInput").ap()
    w2T = nc.dram_tensor("w2T", [128, DCH * NFB * 128], BF16,
                         kind="ExternalInput").ap()
    outT = nc.dram_tensor("outT", [D_MODEL, S_W], BF16, kind="ExternalOutput").ap()
    if DEBUG_TAPS:
        dbg = {
            "dbg_q": nc.dram_tensor("dbg_q", [128, S_W], BF16,
                                    kind="ExternalOutput").ap(),
            "dbg_k": nc.dram_tensor("dbg_k", [128, S_W], BF16,
                                    kind="ExternalOutput").ap(),
            "dbg_v65": nc.dram_tensor("dbg_v65", [128, NSKB * 130], BF16,
                                      kind="ExternalOutput").ap(),
            "dbg_self": nc.dram_tensor("dbg_self", [128, S_W], BF16,
                                       kind="ExternalOutput").ap(),
            "dbg_qc": nc.dram_tensor("dbg_qc", [128, S_W], BF16,
                                     kind="ExternalOutput").ap(),
            "dbg_kc": nc.dram_tensor("dbg_kc", [128, S_P], BF16,
                                     kind="ExternalOutput").ap(),
            "dbg_vc65": nc.dram_tensor("dbg_vc65", [128, NSPB * 130], BF16,
                                       kind="ExternalOutput").ap(),
            "dbg_cross": nc.dram_tensor("dbg_cross", [128, S_W], BF16,
                                        kind="ExternalOutput").ap(),
        }

    rg = [list(range(N_CORES))]

    with (
        tc.tile_pool(name="const", bufs=1) as constp,
        tc.tile_pool(name="dram", bufs=1, space="DRAM") as dramp,
        tc.tile_pool(name="big", bufs=1) as bigp,
        tc.tile_pool(name="work", bufs=2) as workp,
        tc.tile_pool(name="chunk", bufs=2) as chkp,
        tc.tile_pool(name="ps_pp", bufs=2, space="PSUM") as ps_pp,
        tc.tile_pool(name="ps_s", bufs=2, space="PSUM") as ps_s,
        tc.tile_pool(name="ps_o", bufs=1, space="PSUM") as ps_o,
    ):
        # ---- comm warmup: a tiny collective issued first absorbs the
        # global-comm init barrier + first-trigger delay so the first real
        # AllGather starts promptly ----
        warm_sb = constp.tile([128, 1], BF16, tag="warm", name="warm")
        nc.gpsimd.memset(warm_sb[:], 0.0)
        warm_b = dramp.tile([128, 1], BF16, name="warm_b")
        warm_g = dramp.tile([N_CORES * 128, 1], BF16, name="warm_g",
                            addr_space="Shared")
        nc.gpsimd.dma_start(warm_b[:], warm_sb[:])
        nc.gpsimd.collective_compute(
            "AllGather", mybir.AluOpType.bypass, replica_groups=rg,
            ins=[warm_b[:].opt()], outs=[warm_g[:].opt()])

        # ---- resident weight/embedding loads, spread over 3 DMA queues
        # (issue order within a queue = priority) ----
        def load_res(dram_ap, cols, tag, n_split=1, eng=None):
            t = constp.tile([128, cols], BF16, tag=tag, name=tag)
            step = cols // n_split
            for i in range(n_split):
                (eng or nc.sync).dma_start(t[:, step * i:step * (i + 1)],
                                           dram_ap[:, step * i:step * (i + 1)])
            return t

        wq_sb = load_res(wqmT, DCH * HD, "wq")
        wk_sb = load_res(wkmT, DCH * HD, "wk")
        wv_sb = load_res(wvmT, DCH * HD, "wv")
        wemb_sb = load_res(wembT, NSQ * DCH * NC, "wemb", n_split=4)
        wkc_sb = load_res(wkcT, DCH * HD, "wkc", eng=nc.scalar)
        wvc_sb = load_res(wvcT, DCH * HD, "wvc", eng=nc.scalar)
        pemb_sb = load_res(pembT, NSC * DCH * NC, "pemb", n_split=2,
                           eng=nc.scalar)
        wqc_sb = load_res(wqcT, DCH * HD, "wqc", eng=nc.scalar)
        w1_sb = load_res(w1T, DCH * FF_SH, "w1", eng=nc.gpsimd)
        w2_sb = load_res(w2T, DCH * NFB * 128, "w2", eng=nc.gpsimd)

        def xs(c, dc):
            return wemb_sb[:, (c * DCH + dc) * NC:(c * DCH + dc + 1) * NC]

        def xp(sc, dc):
            return pemb_sb[:, (sc * DCH + dc) * NC:(sc * DCH + dc + 1) * NC]

        # ---- activation state ----
        qT = bigp.tile([128, S_W], BF16, tag="qT", name="qT")
        kT = bigp.tile([128, S_W], BF16, tag="kT", name="kT")
        v65 = bigp.tile([128, NSKB * 130], BF16, tag="v65", name="v65")
        kcT = bigp.tile([128, S_P], BF16, tag="kcT", name="kcT")
        vc65 = bigp.tile([128, NSPB * 130], BF16, tag="vc65", name="vc65")
        # ones columns (65th of every 130-block) set once; data copies leave them
        nc.gpsimd.memset(v65[:], 1.0)
        nc.gpsimd.memset(vc65[:], 1.0)

        # ---- projection helpers ----
        def proj_std(out_ap, w_sb, xfun):
            ps = ps_pp.tile([128, NC], F32, tag="pp", name="ps_pj")
            for dc in range(DCH):
                nc.tensor.matmul(
                    ps[:], w_sb[:, HD * dc:HD * (dc + 1)], xfun(dc),
                    start=(dc == 0), stop=(dc == DCH - 1))
            nc.vector.tensor_copy(out_ap, ps[:])

        def proj_v_block(v65_sb, w_sb, xfun, lb, b):
            # out [128 keys, 128 vdims(2 heads)] = x_block^T @ wv
            ps = ps_pp.tile([128, NC], F32, tag="pp", name="ps_pv")
            for dc in range(DCH):
                nc.tensor.matmul(
                    ps[:, 0:128],
                    xfun(dc)[:, 128 * lb:128 * (lb + 1)],
                    w_sb[:, HD * dc:HD * (dc + 1)],
                    start=(dc == 0), stop=(dc == DCH - 1))
            nc.vector.tensor_copy(v65_sb[:, 130 * b:130 * b + 64], ps[:, 0:64])
            nc.vector.tensor_copy(
                v65_sb[:, 130 * b + 65:130 * b + 129], ps[:, 64:128])

        # ---- attention chunk ----
        # Scores for both heads go into one [128, 1024] PSUM pair (adjacent
        # K=64 row-tiles, concurrent), one exp over the live range, AV runs
        # per 128-query sub-block (m=65 with ones-column denominator row).
        # Causal chunks trim dead query ranges on the diagonal j-steps.
        def attention_chunk(out_c, q_ap, k_sb, v65_sb, n_j, causal_c,
                            fillers=()):
            fill = iter(fillers)
            pso = [ps_o.tile([65, NC], F32, tag=f"o{h}", name=f"pso{h}")
                   for h in range(2)]
            for j in range(n_j):
                dk = j - 4 * causal_c if causal_c is not None else None
                if dk is not None and dk < 0:
                    dk = None
                s0 = 128 * dk if dk else 0
                pss = ps_s.tile([128, 2 * NC], F32, tag="s", name="pss")
                for h in range(2):
                    nc.tensor.matmul(
                        pss[:, NC * h + s0:NC * (h + 1)],
                        k_sb[64 * h:64 * (h + 1), 128 * j:128 * (j + 1)],
                        q_ap[64 * h:64 * (h + 1), s0:],
                        start=True, stop=True,
                        tile_position=(64 * h, 0),
                    )
                es = workp.tile([128, 2 * NC], BF16, tag="e", name="es")
                if s0 == 0:
                    nc.scalar.activation(es[:], pss[:], AF.Exp, scale=0.125)
                else:
                    for h in range(2):
                        nc.scalar.activation(
                            es[:, NC * h + s0:NC * (h + 1)],
                            pss[:, NC * h + s0:NC * (h + 1)],
                            AF.Exp, scale=0.125)
                if dk is not None:
                    # partial sub-block [128*dk, 128*dk+128): keep x >= t
                    for h in range(2):
                        nc.gpsimd.affine_select(
                            out=es[:, NC * h + 128 * dk:NC * h + 128 * (dk + 1)],
                            in_=es[:, NC * h + 128 * dk:NC * h + 128 * (dk + 1)],
                            compare_op=mybir.AluOpType.is_ge,
                            fill=0.0,
                            base=0,
                            pattern=[[1, 128]],
                            channel_multiplier=-1,
                        )
                # PSUM zero-regions are bank-granular (2KB): exactly one
                # start (first write marks the bank pending-zero; later
                # sub-blocks' first writes lazily zero) and one stop (last
                # write) per pso bank.
                for h in range(2):
                    for sb in range(dk or 0, 4):
                        nc.tensor.matmul(
                            pso[h][:, 128 * sb:128 * (sb + 1)],
                            v65_sb[:, 130 * j + 65 * h:130 * j + 65 * h + 65],
                            es[:, NC * h + 128 * sb:NC * h + 128 * (sb + 1)],
                            start=(j == 0 and sb == (dk or 0)),
                            stop=(j == n_j - 1 and sb == 3),
                            skip_group_check=True,
                        )
                for th in (next(fill, None),):
                    if th is not None:
                        th()
            for th in fill:
                if th is not None:
                    th()
            for h in range(2):
                lrow = workp.tile([1, NC], F32, tag="lrow", name="lrow")
                nc.vector.tensor_copy(lrow[:], pso[h][64:65, :])
                rec = workp.tile([1, NC], F32, tag="rec", name="rec")
                nc.vector.reciprocal_approx_fast(rec[:], lrow[:])
                rec64 = workp.tile([64, NC], F32, tag="rec64", name="rec64")
                nc.gpsimd.partition_broadcast(rec64[:], rec[:])
                nc.vector.tensor_mul(
                    out_c[64 * h:64 * (h + 1), :], pso[h][0:64, :], rec64[:])

        # ---- collectives: per-chunk AllGather [128, 512] -> [1024, 512] ----
        def allgather(src_sb, name):
            bounce = dramp.tile([128, NC], BF16, name=f"bnc_{name}")
            gath = dramp.tile([N_CORES * 128, NC], BF16, name=f"gd_{name}",
                              addr_space="Shared")
            nc.sync.dma_start(bounce[:], src_sb[:])
            nc.gpsimd.collective_compute(
                "AllGather",
                mybir.AluOpType.bypass,
                replica_groups=rg,
                ins=[bounce[:].opt()],
                outs=[gath[:].opt()],
            )
            return gath

        wd_c = {}
        cd_c = {}
        qc_t = {}
        qc_in = {}
        ffn_state = {}

        # ---- filler work units ----
        def kc_proj(sc):
            proj_std(kcT[:, NC * sc:NC * (sc + 1)], wkc_sb,
                     lambda dc: xp(sc, dc))

        def vc_proj(sc, half):
            for lb in (0, 1) if half == 0 else (2, 3):
                proj_v_block(vc65, wvc_sb, lambda dc: xp(sc, dc),
                             lb, 4 * sc + lb)

        def qc_load(c):
            t = chkp.tile([128, DCH * NC], BF16, tag="wdcat", name=f"word_{c}")
            for dc in range(DCH):
                nc.sync.dma_start(
                    t[:, NC * dc:NC * (dc + 1)],
                    wd_c[c][128 * dc:128 * (dc + 1), :])
            qc_in[c] = t

        def qc_mm(c):
            t = qc_in[c]
            qc = chkp.tile([128, NC], BF16, tag=f"qc{c % 2}", name=f"qcT{c}")
            proj_std(qc[:], wqc_sb,
                     lambda dc: t[:, NC * dc:NC * (dc + 1)])
            qc_t[c] = qc

        def ffn_load(c):
            t = chkp.tile([128, DCH * NC], BF16, tag="xcat", name=f"cr_{c}",
                          bufs=3)
            for dc in range(DCH):
                nc.sync.dma_start(
                    t[:, NC * dc:NC * (dc + 1)],
                    cd_c[c][128 * dc:128 * (dc + 1), :])
            ffn_state[c] = (t, [])

        def ffn1(c, fb):
            t, hts = ffn_state[c]
            ps = ps_pp.tile([128, NC], F32, tag="pp", name="ps_f1")
            for dc in range(DCH):
                nc.tensor.matmul(
                    ps[:],
                    w1_sb[:, FF_SH * dc + 128 * fb:FF_SH * dc + 128 * (fb + 1)],
                    t[:, NC * dc:NC * (dc + 1)],
                    start=(dc == 0), stop=(dc == DCH - 1))
            ht = chkp.tile([128, NC], BF16, tag=f"h{fb}", name=f"hT{fb}_{c}",
                           bufs=2)
            nc.scalar.activation(ht[:], ps[:], AF.Relu)
            hts.append(ht)

        def ffn2(c, ob):
            # alternate PSUM between the pp pool and a borrowed scores-pool
            # bank, and the copy-out between Scalar and Vector: 4 psum bufs
            # in flight, no bank-recycle bubbles on the PE
            hts = ffn_state[c][1]
            if ob % 2 == 0:
                ps = ps_pp.tile([128, NC], F32, tag="pp", name="ps_f2")[:]
            else:
                ps = ps_s.tile([128, 2 * NC], F32, tag="s",
                               name="ps_f2b")[:, 0:NC]
            for fc in range(NFB):
                nc.tensor.matmul(
                    ps,
                    w2_sb[:, NFB * 128 * ob + 128 * fc:NFB * 128 * ob + 128 * (fc + 1)],
                    hts[fc][:],
                    start=(fc == 0), stop=(fc == NFB - 1))
            o_sb = workp.tile([128, NC], BF16, tag="o_sb", name="o_sb", bufs=3)
            if ob % 2 == 0:
                nc.scalar.activation(o_sb[:], ps, AF.Copy)
            else:
                nc.vector.tensor_copy(o_sb[:], ps)
            nc.gpsimd.dma_start(
                outT[128 * ob:128 * (ob + 1), NC * c:NC * (c + 1)], o_sb[:])

        # ---- the pipeline ----
        # self phase: per-chunk qkv projections + attention + AllGather.
        # pemb projections and cross-q projections ride as fillers.
        def self_fillers(c):
            if c == 1:
                return [lambda: kc_proj(0), lambda: vc_proj(0, 0),
                        lambda: vc_proj(0, 1)]
            if c == 2:
                # qc0 needs AGs0 (first AG completes ~mid-c2): load late,
                # matmul last so the in-order PE queue never stalls on it
                return [lambda: kc_proj(1), lambda: vc_proj(1, 0),
                        lambda: vc_proj(1, 1), None, None, None,
                        lambda: qc_load(0), None, None, None, None,
                        lambda: qc_mm(0)]
            if c == 3:
                return [lambda: qc_load(1), None, None,
                        lambda: qc_mm(1),
                        None, None, None, None, None,
                        lambda: qc_load(2), None, None, None,
                        lambda: qc_mm(2)]
            return []

        for c in range(NSQ):
            proj_std(qT[:, NC * c:NC * (c + 1)], wq_sb, lambda dc: xs(c, dc))
            proj_std(kT[:, NC * c:NC * (c + 1)], wk_sb, lambda dc: xs(c, dc))
            for lb in range(4):
                proj_v_block(v65, wv_sb, lambda dc: xs(c, dc), lb, 4 * c + lb)
            out_c = chkp.tile([128, NC], BF16, tag=f"oa{c % 2}",
                              name=f"selfO{c}")
            attention_chunk(out_c[:], qT[:, NC * c:NC * (c + 1)], kT, v65,
                            4 * (c + 1), causal_c=c, fillers=self_fillers(c))
            if DEBUG_TAPS:
                nc.scalar.dma_start(dbg["dbg_self"][:, NC * c:NC * (c + 1)],
                                    out_c[:])
            wd_c[c] = allgather(out_c, f"w{c}")

        # cross phase
        def cross_fillers(c):
            if c == 1:
                # AGs3 (qc3's input) completes ~mid cross c1
                return [lambda: qc_load(3), None, None, lambda: qc_mm(3)]
            if c == 2:
                return [lambda: ffn_load(0)]
            if c == 3:
                return [lambda: ffn_load(1)]
            return []

        for c in range(NSQ):
            out_c = chkp.tile([128, NC], BF16, tag=f"oa{c % 2}",
                              name=f"crossO{c}")
            attention_chunk(out_c[:], qc_t[c][:], kcT, vc65, NSPB,
                            causal_c=None, fillers=cross_fillers(c))
            if DEBUG_TAPS:
                nc.scalar.dma_start(dbg["dbg_cross"][:, NC * c:NC * (c + 1)],
                                    out_c[:])
                nc.scalar.dma_start(dbg["dbg_qc"][:, NC * c:NC * (c + 1)],
                                    qc_t[c][:])
            cd_c[c] = allgather(out_c, f"c{c}")

        if DEBUG_TAPS:
            nc.scalar.dma_start(dbg["dbg_q"][:], qT[:])
            nc.scalar.dma_start(dbg["dbg_k"][:], kT[:])
            nc.scalar.dma_start(dbg["dbg_v65"][:], v65[:])
            nc.scalar.dma_start(dbg["dbg_kc"][:], kcT[:])
            nc.scalar.dma_start(dbg["dbg_vc65"][:], vc65[:])

        # ffn phase (w2 resident; AGc(c) has the whole prior ffn chunks to land)
        for c in range(NSQ):
            if c == 0:
                ffn_load(2)
            if c == 1:
                ffn_load(3)
            for fb in range(NFB):
                ffn1(c, fb)
            for ob in range(DCH):
                ffn2(c, ob)


_CACHED_NC = None


def _build():
    global _CACHED_NC
    if _CACHED_NC is None:
        nc = bacc.Bacc(
            "TRN2",
            target_bir_lowering=False,
            debug=False,
            num_devices=N_CORES,
        )
        with tile.TileContext(nc) as tc:
            decoder_kernel(tc)
        nc.compile()
        _CACHED_NC = nc
    return _CACHED_NC


def _pack_w(wT):
    """[1024, m] -> [128, 8*m]: d-chunk blocks side by side, partition-major."""
    m = wT.shape[1]
    return np.ascontiguousarray(
        wT.reshape(8, 128, m).transpose(1, 0, 2).reshape(128, 8 * m)
    ).astype(ml_dtypes.bfloat16)


def _pack_x(xT, nch):
    """[1024, nch*512] -> [128, nch * 8 * 512]: per seq-chunk c, the 8
    feature-blocks of that chunk's columns, contiguous."""
    return np.ascontiguousarray(
        xT.reshape(8, 128, nch, 512).transpose(1, 2, 0, 3)
        .reshape(128, nch * 8 * 512)).astype(ml_dtypes.bfloat16)


def make_in_maps(inputs):
    """Host-side prep: transposes + per-core weight slices + prepack (bf16)."""
    f = np.ascontiguousarray
    wembT = _pack_x(np.asarray(inputs["wemb"], np.float32).T, NSQ)
    pembT = _pack_x(np.asarray(inputs["pemb"], np.float32).T, NSC)
    in_maps = []
    for i in range(N_CORES):
        hsl = slice(HD * i, HD * (i + 1))
        fsl = slice(FF_SH * i, FF_SH * (i + 1))
        w2T = np.asarray(inputs["W2"], np.float32)[:, fsl].T  # [512, 1024]
        w2h = f(w2T.reshape(4, 128, 8, 128).transpose(1, 2, 0, 3)
                .reshape(128, 4096)).astype(ml_dtypes.bfloat16)
        in_maps.append({
            "wembT": wembT,
            "pembT": pembT,
            "wqmT": _pack_w(np.asarray(inputs["Wq_m"], np.float32)[hsl, :].T),
            "wkmT": _pack_w(np.asarray(inputs["Wk_m"], np.float32)[hsl, :].T),
            "wvmT": _pack_w(np.asarray(inputs["Wv_m"], np.float32)[hsl, :].T),
            "wqcT": _pack_w(np.asarray(inputs["Wq_c"], np.float32)[hsl, :].T),
            "wkcT": _pack_w(np.asarray(inputs["Wk_c"], np.float32)[hsl, :].T),
            "wvcT": _pack_w(np.asarray(inputs["Wv_c"], np.float32)[hsl, :].T),
            "w1T": _pack_w(np.asarray(inputs["W1"], np.float32)[fsl, :].T),
            "w2T": w2h,
        })
    return in_maps


def kernel(**inputs) -> np.ndarray:
    nc = _build()
    in_maps = make_in_maps(inputs)
    res = run_bass_kernel_spmd(nc, in_maps, core_ids=list(range(N_CORES)))
    acc = np.zeros((D_MODEL, S_W), dtype=np.float64)
    for i in range(N_CORES):
        acc += np.asarray(res.results[i]["outT"], dtype=np.float64)
    return np.ascontiguousarray(acc.T.astype(np.float32))
